# revision 1
# baseline (speedup 1.0000x reference)
"""Causal self-attention kernel for 8 Trainium2 NeuronCores.

Problem: B=4, T=2048, C=1024, H=16 heads, HD=64.
  qkv = hs @ qkv_w.T + qkv_b ; per-head causal softmax attention ;
  out = attn @ o_w.T + o_b

Sharding (8 cores): core c handles batch b = c//2 and head-half g = c%2
(8 heads). Each core computes q/k/v for its heads from its batch's
hidden states, runs causal attention, and produces a partial output
projection over its 512 attention-output channels. The host sums the
two partials per batch and adds o_b.

On-device layout/dataflow (per core):
  - host pre-transposes hs and weights so every matmul contraction dim
    lands on SBUF partitions with contiguous DMA lines (no on-device
    transposes).
  - qT, kT stored [d, t] (d on partitions); v stored [t, d] natural,
    augmented with a ones-column so the PV matmul's row 64 accumulates
    the softmax denominator for free.
  - scores computed transposed [j, q] in PSUM; softmax has no
    max-subtraction (scores are ~N(0,1); exp is safe in fp32);
    causal mask applied multiplicatively on the exp'd tile only for
    diagonal-straddling j-chunks.
  - two heads share the PE array via tile_position row packing (K=64)
    and their score tiles sit in one 2-bank PSUM tile so a single
    activation / mask / copy instruction covers the pair (ACT and DVE
    instruction overheads dominate otherwise).
  - all matmuls in float32r (full PE rate, ~1.6e-4 relative rounding).
"""
import numpy as np
from contextlib import ExitStack

import concourse.bass as bass
from concourse import bacc
import concourse.tile as tile
import concourse.mybir as mybir
from concourse.bass_utils import run_bass_kernel_spmd

B, T, C = 4, 2048, 1024
H, HD = 16, 64
NCORES = 8
HPC = H // 2            # 8 heads per core
E = HPC * HD            # 512 local attn-out channels per core
P = 128
SC = 512                # q-chunk (matmul free dim)
NQC = T // SC           # 4 q-chunks
NJC = T // P            # 16 j-chunks
CC = C // P             # 8 contraction chunks
F32 = mybir.dt.float32
F32R = mybir.dt.float32r
Exp = mybir.ActivationFunctionType.Exp
SCALE = HD ** -0.5

_cache = {}


def _build():
    nc = bacc.Bacc("TRN2", target_bir_lowering=False, debug=False)
    hsT = nc.dram_tensor("hsT", [C, T], F32R, kind="ExternalInput")
    wqkvT = nc.dram_tensor("wqkvT", [C, 3 * E], F32R, kind="ExternalInput")
    woT = nc.dram_tensor("woT", [E, C], F32R, kind="ExternalInput")
    bqkv = nc.dram_tensor("bqkv", [P, 8], F32, kind="ExternalInput")
    vbias = nc.dram_tensor("vbias", [P, E], F32, kind="ExternalInput")
    masks = nc.dram_tensor("masks", [P, 1280], mybir.dt.bfloat16, kind="ExternalInput")
    outp = nc.dram_tensor("outp", [T, C], F32, kind="ExternalOutput")

    with tile.TileContext(nc) as tc, ExitStack() as ctx:
        const_pool = ctx.enter_context(tc.tile_pool(name="const", bufs=1))
        qk_pool = ctx.enter_context(tc.tile_pool(name="qk", bufs=1))

        bqkv_sb = const_pool.tile([P, 8], F32)
        vbias_sb = const_pool.tile([P, E], F32)
        masks_sb = const_pool.tile([P, 1280], mybir.dt.bfloat16)
        ones_sb = const_pool.tile([P, 1], F32)
        nc.sync.dma_start(bqkv_sb[:], bqkv.ap())
        nc.vector.memset(ones_sb[:], 1.0)

        kT = qk_pool.tile([P, 4, T], F32R)            # [d%128, d//128, t]
        v_aug = qk_pool.tile([P, NJC, HPC, HD + 1], F32R)  # [t%128, jc, h, d|1]
        nc.vector.tensor_copy(
            v_aug[:, :, :, HD], ones_sb[:, 0, None, None].to_broadcast((P, NJC, HPC))
        )

        # PSUM: 2 x 2-bank rotating tiles (QKV accum pairs / score pairs /
        # o-proj pairs) + 4 x 1-bank PV accumulators = 8 banks.
        ps_all = ctx.enter_context(tc.tile_pool(name="ps", bufs=2, space="PSUM"))
        ps_out = ctx.enter_context(tc.tile_pool(name="pso", bufs=4, space="PSUM"))

        wq_pool = ctx.enter_context(tc.tile_pool(name="wq", bufs=1))
        hst_pool = ctx.enter_context(tc.tile_pool(name="hst", bufs=1))
        qt_pool = ctx.enter_context(tc.tile_pool(name="qtp", bufs=2))
        wo_pool = ctx.enter_context(tc.tile_pool(name="wo", bufs=1))
        attnp_pool = ctx.enter_context(tc.tile_pool(name="attnp", bufs=2))
        exp_pool = ctx.enter_context(tc.tile_pool(name="expp", bufs=3))
        bc_pool = ctx.enter_context(tc.tile_pool(name="bcp", bufs=2))
        rc_pool = ctx.enter_context(tc.tile_pool(name="rcp", bufs=1))
        ost_pool = ctx.enter_context(tc.tile_pool(name="ost", bufs=1))
        qTcs = {}

        wqkvT_sb = wq_pool.tile([P, CC, 3 * E], F32R)
        wq_src = wqkvT.ap().rearrange("(co p) d -> p co d", p=P)
        hst0 = hst_pool.tile([P, CC, SC], F32R, tag="hst", name="hst")
        hs_src0 = hsT.ap()[:, 0:SC].rearrange("(co p) t -> p co t", p=P)
        for cc in range(CC):
            nc.sync.dma_start(wqkvT_sb[:, cc], wq_src[:, cc])
            nc.sync.dma_start(hst0[:, cc], hs_src0[:, cc])
        # non-startup-critical loads, after the phase-1 gating DMAs
        nc.sync.dma_start(vbias_sb[:], vbias.ap())
        nc.sync.dma_start(masks_sb[:], masks.ap())
        woT_sb = wo_pool.tile([P, E // P, C], F32R)
        nc.sync.dma_start(woT_sb[:], woT.ap().rearrange("(ec p) co -> p ec co", p=P))

        def qkv_units(t4):
            st = {}

            def u_load():
                if t4 == 0:
                    st["hst"] = hst0
                else:
                    hst = hst_pool.tile([P, CC, SC], F32R, tag="hst", name="hst")
                    nc.sync.dma_start(
                        hst[:],
                        hsT.ap()[:, t4 * SC:(t4 + 1) * SC].rearrange(
                            "(co p) t -> p co t", p=P
                        ),
                    )
                    st["hst"] = hst
                qTcs[t4] = qt_pool.tile([P, 4, SC], F32R, tag="qTc", name="qTc")

            def u_qk_half(qk, dp, half):
                if "hst" not in st:
                    u_load()
                hst = st["hst"]
                if half == 0:
                    st["ps"] = ps_all.tile([P, 2, SC], F32, tag="ps2", name="psq")
                ps = st["ps"]
                w0 = qk * E + (2 * dp + half) * P
                for cc in range(CC):
                    nc.tensor.matmul(
                        ps[:, half], wqkvT_sb[:, cc, w0:w0 + P], hst[:, cc, :],
                        start=(cc == 0), stop=(cc == CC - 1),
                    )
                if half == 1:
                    dst = (qTcs[t4][:, 2 * dp:2 * dp + 2, :] if qk == 0 else
                           kT[:, 2 * dp:2 * dp + 2, t4 * SC:(t4 + 1) * SC])
                    nc.vector.tensor_add(
                        dst, ps[:],
                        bqkv_sb[:, qk * 4 + 2 * dp:qk * 4 + 2 * dp + 2, None]
                        .to_broadcast((P, 2, SC)),
                    )

            def u_v_half(tp, half):
                hst = st["hst"]
                if half == 0:
                    st["ps"] = ps_all.tile([P, 2, SC], F32, tag="ps2", name="psv")
                ps = st["ps"]
                ts = 2 * tp + half
                for cc in range(CC):
                    nc.tensor.matmul(
                        ps[:, half], hst[:, cc, ts * P:(ts + 1) * P],
                        wqkvT_sb[:, cc, 2 * E:3 * E],
                        start=(cc == 0), stop=(cc == CC - 1),
                    )
                if half == 1:
                    jc = t4 * 4 + 2 * tp
                    nc.vector.tensor_add(
                        v_aug[:, jc:jc + 2, :, 0:HD],
                        ps[:].rearrange("p two (h d) -> p two h d", d=HD),
                        vbias_sb[:, None].rearrange(
                            "p two (h d) -> p two h d", d=HD
                        ).to_broadcast((P, 2, HPC, HD)),
                    )

            units = (
                [lambda qk=qk, dp=dp, h=h: u_qk_half(qk, dp, h)
                 for qk in (0, 1) for dp in (0, 1) for h in (0, 1)]
                + [lambda tp=tp, h=h: u_v_half(tp, h)
                   for tp in (0, 1) for h in (0, 1)]
            )
            return u_load, units

        def emit_qkv(t4):
            u_load, units = qkv_units(t4)
            u_load()
            for u in units:
                u()

        def emit_attn(qc, hp, attnp, step_hook=None):
            q0 = qc * SC
            nj = 4 * (qc + 1)
            out_ps = [
                ps_out.tile([HD + 1, SC], F32, tag="outps", name=f"outps{s}")
                for s in range(2)
            ]

            def emit_pv(item):
                jc, n0, e = item
                for s in range(2):
                    nc.tensor.matmul(
                        out_ps[s][:, n0:SC], v_aug[:, jc, 2 * hp + s, :],
                        e[:, s, n0:SC],
                        start=(jc == 0), stop=(jc == nj - 1),
                    )

            pends = []  # deferred PV pairs (2-deep software pipeline skew)
            for jc in range(nj):
                di = jc - 4 * qc  # >= 0 on diagonal-straddling chunks
                n0 = P * di if di >= 0 else 0
                j0 = jc * P
                sc_ps = ps_all.tile([P, 2, SC], F32, tag="ps2", name="scps")
                for s in range(2):
                    nc.tensor.matmul(
                        sc_ps[:, s, n0:SC],
                        kT[64 * s:64 * s + 64, hp, j0:j0 + P],
                        qTcs[qc][64 * s:64 * s + 64, hp, n0:SC],
                        start=True, stop=True, tile_position=(64 * s, 0),
                    )
                if len(pends) >= 2:
                    emit_pv(pends.pop(0))
                if step_hook is not None:
                    step_hook()
                e = exp_pool.tile([P, 2, SC], F32R, tag="exp")
                nc.scalar.activation(
                    e[:, :, n0:SC], sc_ps[:, :, n0:SC], Exp, scale=SCALE
                )
                if di >= 0:
                    off = (0, 512, 896, 1152)[di]
                    nc.vector.tensor_mul(
                        e[:, :, n0:SC], e[:, :, n0:SC].bitcast(F32),
                        masks_sb[:, None, off:off + SC - n0]
                        .to_broadcast((P, 2, SC - n0)),
                    )
                pends.append((jc, n0, e))
            for item in pends:
                emit_pv(item)
            # normalize by the ones-row sum and place into attnp
            for s in range(2):
                srow = rc_pool.tile([1, SC], F32, tag="srow")
                nc.vector.tensor_copy(srow[:], out_ps[s][HD:HD + 1, :])
                bc = bc_pool.tile([64, SC], F32, tag="bc")
                nc.gpsimd.partition_broadcast(bc[:], srow[:])
                rc = bc_pool.tile([64, SC], F32, tag="rcb")
                nc.vector.reciprocal_approx_fast(rc[:], bc[:])
                nc.vector.tensor_mul(
                    attnp[64 * s:64 * s + 64, hp, :], out_ps[s][0:HD, :], rc[:]
                )

        def oproj_unit(qc, attnp, t8):
            # both co halves in one 2-bank psum tile -> one copy, one
            # contiguous-row DMA
            trow = qc * SC + t8 * P
            po = ps_all.tile([P, 2, SC], F32, tag="ps2", name="psop")
            for co in range(2):
                for ec in range(E // P):
                    nc.tensor.matmul(
                        po[:, co], attnp[:, ec, t8 * P:(t8 + 1) * P],
                        woT_sb[:, ec, co * SC:(co + 1) * SC],
                        start=(ec == 0), stop=(ec == E // P - 1),
                    )
            st = ost_pool.tile([P, 2, SC], F32, tag="ost")
            nc.vector.tensor_copy(st[:], po[:])
            nc.sync.dma_start(outp.ap()[trow:trow + P, :], st[:])

        def emit_oproj(qc, attnp):
            for t8 in range(SC // P):
                oproj_unit(qc, attnp, t8)

        # software-pipelined emission: QKV for chunk t4+1 and the previous
        # q-chunk's output projection are split into 8-matmul micro-units
        # and dripped into the attention j-loop (the PE is in-order, so
        # filler work must sit between attention steps to absorb the
        # ACT-paced exp lag).
        u_load0, units0 = qkv_units(0)
        u_load0()
        for u in units0[:8]:
            u()  # q and k of chunk 0; the v units ride inside qc0's stream
        carry = units0[8:]
        attnps = {}
        for qc in range(NQC):
            attnps[qc] = attnp_pool.tile(
                [P, E // P, SC], F32R, tag="attnp", name="attnp"
            )
            fillers = list(carry)
            carry = []
            if qc + 1 < NQC:
                u_load, units = qkv_units(qc + 1)
                u_load()  # issue the hsT chunk DMA as early as possible
                fillers.extend(units)
            if qc > 0:
                fillers.extend(
                    (lambda t8=t8: oproj_unit(qc - 1, attnps[qc - 1], t8))
                    for t8 in range(SC // P)
                )
            steps_total = 4 * 4 * (qc + 1)
            nun = len(fillers)
            # start fillers a few steps in: the hoisted hsT DMA needs ~4us
            # (it waits for the previous chunk's last reader) before the
            # first QKV filler's matmuls can run
            state = {"step": 0, "done": 0}

            def step_hook():
                state["step"] += 1
                while (state["done"] < nun
                       and state["step"] * nun >= (state["done"] + 1) * steps_total):
                    fillers[state["done"]]()
                    state["done"] += 1

            for hp in range(4):
                emit_attn(qc, hp, attnps[qc], step_hook)
            for u in fillers[state["done"]:]:
                u()
        emit_oproj(NQC - 1, attnps[NQC - 1])

    nc.compile()
    return nc


def _prep_inputs(hidden_states, qkv_w, qkv_b, o_w, o_b):
    hidden_states = np.asarray(hidden_states, dtype=np.float32)
    qkv_w = np.asarray(qkv_w, dtype=np.float32)
    qkv_b = np.asarray(qkv_b, dtype=np.float32)
    o_w = np.asarray(o_w, dtype=np.float32)

    import ml_dtypes
    msk = np.zeros((P, 1280), dtype=ml_dtypes.bfloat16)
    j = np.arange(P)[:, None]
    offs = (0, 512, 896, 1152)
    for i in range(4):
        n0 = P * i
        q = np.arange(n0, SC)[None, :]
        msk[:, offs[i]:offs[i] + SC - n0] = ((P * i + j) <= q).astype(ml_dtypes.bfloat16)

    in_maps = []
    for c in range(NCORES):
        b, g = c // 2, c % 2
        hsT = np.ascontiguousarray(hidden_states[b].T)
        qsel = qkv_w[E * g:E * g + E]
        ksel = qkv_w[C + E * g:C + E * g + E]
        vsel = qkv_w[2 * C + E * g:2 * C + E * g + E]
        wqkvT = np.ascontiguousarray(np.concatenate([qsel, ksel, vsel], 0).T)
        woT = np.ascontiguousarray(o_w[:, E * g:E * g + E].T)
        bq = qkv_b[E * g:E * g + E].reshape(4, P).T
        bk = qkv_b[C + E * g:C + E * g + E].reshape(4, P).T
        bv = qkv_b[2 * C + E * g:2 * C + E * g + E]
        bqkv = np.ascontiguousarray(np.concatenate([bq, bk], 1))
        vbias = np.ascontiguousarray(np.tile(bv[None, :], (P, 1)))
        in_maps.append({
            "hsT": hsT, "wqkvT": wqkvT, "woT": woT,
            "bqkv": bqkv, "vbias": vbias, "masks": msk,
        })
    return in_maps


def _get_nc():
    if "nc" not in _cache:
        _cache["nc"] = _build()
    return _cache["nc"]


def _run(in_maps, **kwargs):
    return run_bass_kernel_spmd(
        _get_nc(), in_maps, core_ids=list(range(NCORES)), **kwargs
    )


def kernel(hidden_states, qkv_w, qkv_b, o_w, o_b, **_):
    in_maps = _prep_inputs(hidden_states, qkv_w, qkv_b, o_w, o_b)
    res = _run(in_maps)
    o_b = np.asarray(o_b, dtype=np.float32)
    out = np.empty((B, T, C), dtype=np.float32)
    for b in range(B):
        out[b] = res.results[2 * b]["outp"] + res.results[2 * b + 1]["outp"] + o_b
    return out



# revision 9
# speedup vs baseline: 1.0751x; 1.0751x over previous
"""Causal self-attention kernel for 8 Trainium2 NeuronCores.

Problem: B=4, T=2048, C=1024, H=16 heads, HD=64.
  qkv = hs @ qkv_w.T + qkv_b ; per-head causal softmax attention ;
  out = attn @ o_w.T + o_b

Sharding (8 cores): core c handles batch b = c//2 and head-half g = c%2
(8 heads). Each core computes q/k/v for its heads from its batch's
hidden states, runs causal attention, and produces a partial output
projection over its 512 attention-output channels. The host sums the
two partials per batch and adds o_b.

On-device layout/dataflow (per core):
  - host pre-transposes hs and weights so every matmul contraction dim
    lands on SBUF partitions with contiguous DMA lines (no on-device
    transposes).
  - qT, kT stored [d, t] (d on partitions); v stored [t, d] natural,
    augmented with a ones-column so the PV matmul's row 64 accumulates
    the softmax denominator for free.
  - scores computed transposed [j, q] in PSUM; softmax has no
    max-subtraction (scores are ~N(0,1); exp is safe in fp32);
    causal mask applied multiplicatively on the exp'd tile only for
    diagonal-straddling j-chunks.
  - two heads share the PE array via tile_position row packing (K=64)
    and their score tiles sit in one 2-bank PSUM tile so a single
    activation / mask / copy instruction covers the pair (ACT and DVE
    instruction overheads dominate otherwise).
  - all matmuls in float32r (full PE rate, ~1.6e-4 relative rounding).
"""
import numpy as np
from contextlib import ExitStack

import concourse.bass as bass
from concourse import bacc
import concourse.tile as tile
import concourse.mybir as mybir
from concourse.bass_utils import run_bass_kernel_spmd

B, T, C = 4, 2048, 1024
H, HD = 16, 64
NCORES = 8
HPC = H // 2            # 8 heads per core
E = HPC * HD            # 512 local attn-out channels per core
P = 128
SC = 512                # q-chunk (matmul free dim)
NQC = T // SC           # 4 q-chunks
NJC = T // P            # 16 j-chunks
CC = C // P             # 8 contraction chunks
F32 = mybir.dt.float32
F32R = mybir.dt.float32r
BF16 = mybir.dt.bfloat16
Exp = mybir.ActivationFunctionType.Exp
SCALE = HD ** -0.5

_cache = {}


def _build():
    nc = bacc.Bacc("TRN2", target_bir_lowering=False, debug=False)
    hsT = nc.dram_tensor("hsT", [C, T], BF16, kind="ExternalInput")
    wqkvT = nc.dram_tensor("wqkvT", [C, 3 * E], BF16, kind="ExternalInput")
    woT = nc.dram_tensor("woT", [E, C], F32R, kind="ExternalInput")
    bqkv = nc.dram_tensor("bqkv", [P, 8], F32, kind="ExternalInput")
    vbias = nc.dram_tensor("vbias", [P, E], F32, kind="ExternalInput")
    masks = nc.dram_tensor("masks", [P, 1280], mybir.dt.bfloat16, kind="ExternalInput")
    outp = nc.dram_tensor("outp", [T, C], F32, kind="ExternalOutput")

    with tile.TileContext(nc) as tc, ExitStack() as ctx:
        const_pool = ctx.enter_context(tc.tile_pool(name="const", bufs=1))
        qk_pool = ctx.enter_context(tc.tile_pool(name="qk", bufs=1))

        bqkv_sb = const_pool.tile([P, 8], F32)
        vbias_sb = const_pool.tile([P, E], F32)
        masks_sb = const_pool.tile([P, 1280], mybir.dt.bfloat16)
        ones_sb = const_pool.tile([P, 1], F32)
        nc.sync.dma_start(bqkv_sb[:], bqkv.ap())
        nc.vector.memset(ones_sb[:], 1.0)

        kT = qk_pool.tile([P, 4, T], F32R)            # [d%128, d//128, t]
        v_aug = qk_pool.tile([P, NJC, HPC, HD + 1], F32R)  # [t%128, jc, h, d|1]
        nc.vector.tensor_copy(
            v_aug[:, :, :, HD], ones_sb[:, 0, None, None].to_broadcast((P, NJC, HPC))
        )

        # PSUM: 2 x 2-bank rotating tiles (QKV accum pairs / score pairs /
        # o-proj pairs) + 4 x 1-bank PV accumulators = 8 banks.
        ps_all = ctx.enter_context(tc.tile_pool(name="ps", bufs=2, space="PSUM"))
        ps_out = ctx.enter_context(tc.tile_pool(name="pso", bufs=4, space="PSUM"))

        wq_pool = ctx.enter_context(tc.tile_pool(name="wq", bufs=1))
        hst_pool = ctx.enter_context(tc.tile_pool(name="hst", bufs=2))
        qt_pool = ctx.enter_context(tc.tile_pool(name="qtp", bufs=2))
        wo_pool = ctx.enter_context(tc.tile_pool(name="wo", bufs=1))
        attnp_pool = ctx.enter_context(tc.tile_pool(name="attnp", bufs=2))
        exp_pool = ctx.enter_context(tc.tile_pool(name="expp", bufs=3))
        bc_pool = ctx.enter_context(tc.tile_pool(name="bcp", bufs=2))
        rc_pool = ctx.enter_context(tc.tile_pool(name="rcp", bufs=1))
        ost_pool = ctx.enter_context(tc.tile_pool(name="ost", bufs=1))
        qTcs = {}

        # scratch for PE warm-up + ACT exp-table preload, both issued into
        # the kernel-entry dead time (first DMA can't even issue before
        # ~8us of framework barrier/setup).
        scratch = const_pool.tile([P, SC], F32)
        nc.vector.memset(scratch[:], 0.0)
        dummy_act = const_pool.tile([1, 1], F32)
        nc.scalar.activation(dummy_act[:], ones_sb[0:1, 0:1], Exp, scale=1.0)

        # startup DMAs, ordered so the first QKV unit's data lands first:
        # hsT chunk 0, then the q/k weight columns in 128-col blocks (block
        # b gates only unit b), then the v columns / other constants.
        wqkvT_sb = wq_pool.tile([P, CC, 3 * E], BF16)
        hst0 = hst_pool.tile([P, CC, SC], BF16, tag="hst", name="hst")
        nc.sync.dma_start(
            hst0[:], hsT.ap()[:, 0:SC].rearrange("(co p) t -> p co t", p=P)
        )
        for b8 in range(2 * E // P):
            nc.sync.dma_start(
                wqkvT_sb[:, :, b8 * P:(b8 + 1) * P],
                wqkvT.ap()[:, b8 * P:(b8 + 1) * P].rearrange(
                    "(co p) d -> p co d", p=P
                ),
            )
        nc.sync.dma_start(
            wqkvT_sb[:, :, 2 * E:3 * E],
            wqkvT.ap()[:, 2 * E:3 * E].rearrange("(co p) d -> p co d", p=P),
        )
        # PE warm-up: ~16 matmuls on zero scratch lift the HAM clock gate
        # (4/8 -> 8/8 after ~3.4us of sustained activity) while the real
        # stream is still DMA-gated, so the first QKV unit runs at 2.4GHz.
        wps = ps_all.tile([P, 2, SC], F32, tag="ps2", name="warm")
        for r in range(16):
            nc.tensor.matmul(
                wps[:, r % 2], scratch[:, 0:P].bitcast(F32R),
                scratch[:].bitcast(F32R), start=True, stop=True,
            )
        # non-startup-critical loads, after the phase-1 gating DMAs
        nc.sync.dma_start(vbias_sb[:], vbias.ap())
        nc.sync.dma_start(masks_sb[:], masks.ap())
        woT_sb = wo_pool.tile([P, E // P, C], F32R)
        nc.sync.dma_start(woT_sb[:], woT.ap().rearrange("(ec p) co -> p ec co", p=P))

        def qkv_units(t4):
            st = {}

            def u_load():
                if t4 == 0:
                    st["hst"] = hst0
                else:
                    hst = hst_pool.tile([P, CC, SC], BF16, tag="hst", name="hst")
                    nc.sync.dma_start(
                        hst[:],
                        hsT.ap()[:, t4 * SC:(t4 + 1) * SC].rearrange(
                            "(co p) t -> p co t", p=P
                        ),
                    )
                    st["hst"] = hst
                qTcs[t4] = qt_pool.tile([P, 4, SC], F32R, tag="qTc", name="qTc")

            def u_qk_half(qk, dp, half):
                if "hst" not in st:
                    u_load()
                hst = st["hst"]
                if half == 0:
                    st["ps"] = ps_all.tile([P, 2, SC], F32, tag="ps2", name="psq")
                ps = st["ps"]
                w0 = qk * E + (2 * dp + half) * P
                for cc in range(CC):
                    nc.tensor.matmul(
                        ps[:, half], wqkvT_sb[:, cc, w0:w0 + P], hst[:, cc, :],
                        start=(cc == 0), stop=(cc == CC - 1),
                    )
                if half == 1:
                    dst = (qTcs[t4][:, 2 * dp:2 * dp + 2, :] if qk == 0 else
                           kT[:, 2 * dp:2 * dp + 2, t4 * SC:(t4 + 1) * SC])
                    nc.vector.tensor_add(
                        dst, ps[:],
                        bqkv_sb[:, qk * 4 + 2 * dp:qk * 4 + 2 * dp + 2, None]
                        .to_broadcast((P, 2, SC)),
                    )

            def u_v_half(tp, half):
                hst = st["hst"]
                if half == 0:
                    st["ps"] = ps_all.tile([P, 2, SC], F32, tag="ps2", name="psv")
                ps = st["ps"]
                ts = 2 * tp + half
                for cc in range(CC):
                    nc.tensor.matmul(
                        ps[:, half], hst[:, cc, ts * P:(ts + 1) * P],
                        wqkvT_sb[:, cc, 2 * E:3 * E],
                        start=(cc == 0), stop=(cc == CC - 1),
                    )
                if half == 1:
                    jc = t4 * 4 + 2 * tp
                    nc.vector.tensor_add(
                        v_aug[:, jc:jc + 2, :, 0:HD],
                        ps[:].rearrange("p two (h d) -> p two h d", d=HD),
                        vbias_sb[:, None].rearrange(
                            "p two (h d) -> p two h d", d=HD
                        ).to_broadcast((P, 2, HPC, HD)),
                    )

            units = (
                [lambda qk=qk, dp=dp, h=h: u_qk_half(qk, dp, h)
                 for qk in (0, 1) for dp in (0, 1) for h in (0, 1)]
                + [lambda tp=tp, h=h: u_v_half(tp, h)
                   for tp in (0, 1) for h in (0, 1)]
            )
            return u_load, units

        def emit_qkv(t4):
            u_load, units = qkv_units(t4)
            u_load()
            for u in units:
                u()

        def emit_attn(qc, hp, attnp, step_hook=None):
            q0 = qc * SC
            nj = 4 * (qc + 1)
            out_ps = [
                ps_out.tile([HD + 1, SC], F32, tag="outps", name=f"outps{s}")
                for s in range(2)
            ]

            def emit_pv(item):
                jc, n0, e = item
                for s in range(2):
                    nc.tensor.matmul(
                        out_ps[s][:, n0:SC], v_aug[:, jc, 2 * hp + s, :],
                        e[:, s, n0:SC],
                        start=(jc == 0), stop=(jc == nj - 1),
                    )

            pends = []  # deferred PV pairs (2-deep software pipeline skew)
            for jc in range(nj):
                di = jc - 4 * qc  # >= 0 on diagonal-straddling chunks
                n0 = P * di if di >= 0 else 0
                j0 = jc * P
                sc_ps = ps_all.tile([P, 2, SC], F32, tag="ps2", name="scps")
                for s in range(2):
                    nc.tensor.matmul(
                        sc_ps[:, s, n0:SC],
                        kT[64 * s:64 * s + 64, hp, j0:j0 + P],
                        qTcs[qc][64 * s:64 * s + 64, hp, n0:SC],
                        start=True, stop=True, tile_position=(64 * s, 0),
                    )
                if len(pends) >= 2:
                    emit_pv(pends.pop(0))
                if step_hook is not None:
                    step_hook()
                e = exp_pool.tile([P, 2, SC], F32R, tag="exp")
                nc.scalar.activation(
                    e[:, :, n0:SC], sc_ps[:, :, n0:SC], Exp, scale=SCALE
                )
                if di >= 0:
                    off = (0, 512, 896, 1152)[di]
                    nc.vector.tensor_mul(
                        e[:, :, n0:SC], e[:, :, n0:SC].bitcast(F32),
                        masks_sb[:, None, off:off + SC - n0]
                        .to_broadcast((P, 2, SC - n0)),
                    )
                pends.append((jc, n0, e))
            for item in pends:
                emit_pv(item)
            # normalize by the ones-row sum and place into attnp
            for s in range(2):
                srow = rc_pool.tile([1, SC], F32, tag="srow")
                nc.vector.tensor_copy(srow[:], out_ps[s][HD:HD + 1, :])
                bc = bc_pool.tile([64, SC], F32, tag="bc")
                nc.gpsimd.partition_broadcast(bc[:], srow[:])
                rc = bc_pool.tile([64, SC], F32, tag="rcb")
                nc.vector.reciprocal_approx_fast(rc[:], bc[:])
                nc.vector.tensor_mul(
                    attnp[64 * s:64 * s + 64, hp, :], out_ps[s][0:HD, :], rc[:]
                )

        def oproj_unit(qc, attnp, t8):
            # both co halves in one 2-bank psum tile -> one copy, one
            # contiguous-row DMA
            trow = qc * SC + t8 * P
            po = ps_all.tile([P, 2, SC], F32, tag="ps2", name="psop")
            for co in range(2):
                for ec in range(E // P):
                    nc.tensor.matmul(
                        po[:, co], attnp[:, ec, t8 * P:(t8 + 1) * P],
                        woT_sb[:, ec, co * SC:(co + 1) * SC],
                        start=(ec == 0), stop=(ec == E // P - 1),
                    )
            st = ost_pool.tile([P, 2, SC], F32, tag="ost")
            nc.vector.tensor_copy(st[:], po[:])
            nc.sync.dma_start(outp.ap()[trow:trow + P, :], st[:])

        def emit_oproj(qc, attnp):
            for t8 in range(SC // P):
                oproj_unit(qc, attnp, t8)

        # software-pipelined emission: QKV for chunk t4+1 and the previous
        # q-chunk's output projection are split into 8-matmul micro-units
        # and dripped into the attention j-loop (the PE is in-order, so
        # filler work must sit between attention steps to absorb the
        # ACT-paced exp lag).
        u_load0, units0 = qkv_units(0)
        u_load0()
        for u in units0[:8]:
            u()  # q and k of chunk 0; the v units ride inside qc0's stream
        carry = units0[8:]
        attnps = {}
        for qc in range(NQC):
            attnps[qc] = attnp_pool.tile(
                [P, E // P, SC], F32R, tag="attnp", name="attnp"
            )
            fillers = list(carry)
            carry = []
            if qc + 1 < NQC:
                u_load, units = qkv_units(qc + 1)
                u_load()  # issue the hsT chunk DMA as early as possible
                fillers.extend(units)
            if qc > 0:
                fillers.extend(
                    (lambda t8=t8: oproj_unit(qc - 1, attnps[qc - 1], t8))
                    for t8 in range(SC // P)
                )
            steps_total = 4 * 4 * (qc + 1)
            nun = len(fillers)
            # start fillers a few steps in: the hoisted hsT DMA needs ~4us
            # (it waits for the previous chunk's last reader) before the
            # first QKV filler's matmuls can run
            state = {"step": 0, "done": 0}

            def step_hook():
                state["step"] += 1
                while (state["done"] < nun
                       and state["step"] * nun >= (state["done"] + 1) * steps_total):
                    fillers[state["done"]]()
                    state["done"] += 1

            for hp in range(4):
                emit_attn(qc, hp, attnps[qc], step_hook)
            for u in fillers[state["done"]:]:
                u()
        emit_oproj(NQC - 1, attnps[NQC - 1])

    nc.compile()
    return nc


def _prep_inputs(hidden_states, qkv_w, qkv_b, o_w, o_b):
    hidden_states = np.asarray(hidden_states, dtype=np.float32)
    qkv_w = np.asarray(qkv_w, dtype=np.float32)
    qkv_b = np.asarray(qkv_b, dtype=np.float32)
    o_w = np.asarray(o_w, dtype=np.float32)

    import ml_dtypes
    msk = np.zeros((P, 1280), dtype=ml_dtypes.bfloat16)
    j = np.arange(P)[:, None]
    offs = (0, 512, 896, 1152)
    for i in range(4):
        n0 = P * i
        q = np.arange(n0, SC)[None, :]
        msk[:, offs[i]:offs[i] + SC - n0] = ((P * i + j) <= q).astype(ml_dtypes.bfloat16)

    in_maps = []
    for c in range(NCORES):
        b, g = c // 2, c % 2
        hsT = np.ascontiguousarray(hidden_states[b].T.astype(ml_dtypes.bfloat16))
        qsel = qkv_w[E * g:E * g + E]
        ksel = qkv_w[C + E * g:C + E * g + E]
        vsel = qkv_w[2 * C + E * g:2 * C + E * g + E]
        wqkvT = np.ascontiguousarray(
            np.concatenate([qsel, ksel, vsel], 0).T.astype(ml_dtypes.bfloat16)
        )
        woT = np.ascontiguousarray(o_w[:, E * g:E * g + E].T)
        bq = qkv_b[E * g:E * g + E].reshape(4, P).T
        bk = qkv_b[C + E * g:C + E * g + E].reshape(4, P).T
        bv = qkv_b[2 * C + E * g:2 * C + E * g + E]
        bqkv = np.ascontiguousarray(np.concatenate([bq, bk], 1))
        vbias = np.ascontiguousarray(np.tile(bv[None, :], (P, 1)))
        in_maps.append({
            "hsT": hsT, "wqkvT": wqkvT, "woT": woT,
            "bqkv": bqkv, "vbias": vbias, "masks": msk,
        })
    return in_maps


def _get_nc():
    if "nc" not in _cache:
        _cache["nc"] = _build()
    return _cache["nc"]


def _run(in_maps, **kwargs):
    return run_bass_kernel_spmd(
        _get_nc(), in_maps, core_ids=list(range(NCORES)), **kwargs
    )


def kernel(hidden_states, qkv_w, qkv_b, o_w, o_b, **_):
    in_maps = _prep_inputs(hidden_states, qkv_w, qkv_b, o_w, o_b)
    res = _run(in_maps)
    o_b = np.asarray(o_b, dtype=np.float32)
    out = np.empty((B, T, C), dtype=np.float32)
    for b in range(B):
        out[b] = res.results[2 * b]["outp"] + res.results[2 * b + 1]["outp"] + o_b
    return out



# revision 13
# speedup vs baseline: 1.1250x; 1.0464x over previous
"""Causal self-attention kernel for 8 Trainium2 NeuronCores.

Problem: B=4, T=2048, C=1024, H=16 heads, HD=64.
  qkv = hs @ qkv_w.T + qkv_b ; per-head causal softmax attention ;
  out = attn @ o_w.T + o_b

Sharding (8 cores): core c handles batch b = c//2 and head-half g = c%2
(8 heads). Each core computes q/k/v for its heads from its batch's
hidden states, runs causal attention, and produces a partial output
projection over its 512 attention-output channels. The host sums the
two partials per batch and adds o_b.

On-device layout/dataflow (per core):
  - host pre-transposes hs and weights so every matmul contraction dim
    lands on SBUF partitions with contiguous DMA lines (no on-device
    transposes).
  - qT, kT stored [d, t] (d on partitions); v stored [t, d] natural,
    augmented with a ones-column so the PV matmul's row 64 accumulates
    the softmax denominator for free.
  - scores computed transposed [j, q] in PSUM; softmax has no
    max-subtraction (scores are ~N(0,1); exp is safe in fp32);
    causal mask applied multiplicatively on the exp'd tile only for
    diagonal-straddling j-chunks.
  - two heads share the PE array via tile_position row packing (K=64)
    and their score tiles sit in one 2-bank PSUM tile so a single
    activation / mask / copy instruction covers the pair (ACT and DVE
    instruction overheads dominate otherwise).
  - all matmuls in float32r (full PE rate, ~1.6e-4 relative rounding).
"""
import numpy as np
from contextlib import ExitStack

import concourse.bass as bass
from concourse import bacc
import concourse.tile as tile
import concourse.mybir as mybir
from concourse.bass_utils import run_bass_kernel_spmd

B, T, C = 4, 2048, 1024
H, HD = 16, 64
NCORES = 8
HPC = H // 2            # 8 heads per core
E = HPC * HD            # 512 local attn-out channels per core
P = 128
SC = 512                # q-chunk (matmul free dim)
NQC = T // SC           # 4 q-chunks
NJC = T // P            # 16 j-chunks
CC = C // P             # 8 contraction chunks
F32 = mybir.dt.float32
F32R = mybir.dt.float32r
BF16 = mybir.dt.bfloat16
Exp = mybir.ActivationFunctionType.Exp
SCALE = HD ** -0.5

_cache = {}


def _build():
    nc = bacc.Bacc("TRN2", target_bir_lowering=False, debug=False)
    hsT = nc.dram_tensor("hsT", [C, T], BF16, kind="ExternalInput")
    wqkvT = nc.dram_tensor("wqkvT", [C, 3 * E], BF16, kind="ExternalInput")
    woT = nc.dram_tensor("woT", [E, C], F32R, kind="ExternalInput")
    bqkv = nc.dram_tensor("bqkv", [P, 8], F32, kind="ExternalInput")
    vbias = nc.dram_tensor("vbias", [P, E], F32, kind="ExternalInput")
    masks = nc.dram_tensor("masks", [P, 1280], mybir.dt.bfloat16, kind="ExternalInput")
    outp = nc.dram_tensor("outp", [T, C], F32, kind="ExternalOutput")

    with tile.TileContext(nc) as tc, ExitStack() as ctx:
        const_pool = ctx.enter_context(tc.tile_pool(name="const", bufs=1))
        qk_pool = ctx.enter_context(tc.tile_pool(name="qk", bufs=1))

        bqkv_sb = const_pool.tile([P, 8], F32)
        vbias_sb = const_pool.tile([P, E], F32)
        masks_sb = const_pool.tile([P, 1280], mybir.dt.bfloat16)
        ones_sb = const_pool.tile([P, 1], F32)
        nc.sync.dma_start(bqkv_sb[:], bqkv.ap())
        nc.vector.memset(ones_sb[:], 1.0)

        kT = qk_pool.tile([P, 4, T], F32R)            # [d%128, d//128, t]
        v_aug = qk_pool.tile([P, NJC, HPC, HD + 1], F32R)  # [t%128, jc, h, d|1]
        nc.vector.tensor_copy(
            v_aug[:, :, :, HD], ones_sb[:, 0, None, None].to_broadcast((P, NJC, HPC))
        )

        # PSUM: 2 x 2-bank rotating tiles (QKV accum pairs / score pairs /
        # o-proj pairs) + 4 x 1-bank PV accumulators = 8 banks.
        ps_all = ctx.enter_context(tc.tile_pool(name="ps", bufs=2, space="PSUM"))
        ps_out = ctx.enter_context(tc.tile_pool(name="pso", bufs=4, space="PSUM"))

        wq_pool = ctx.enter_context(tc.tile_pool(name="wq", bufs=1))
        hst_pool = ctx.enter_context(tc.tile_pool(name="hst", bufs=2))
        qt_pool = ctx.enter_context(tc.tile_pool(name="qtp", bufs=2))
        wo_pool = ctx.enter_context(tc.tile_pool(name="wo", bufs=1))
        attnp_pool = ctx.enter_context(tc.tile_pool(name="attnp", bufs=2))
        exp_pool = ctx.enter_context(tc.tile_pool(name="expp", bufs=3))
        bc_pool = ctx.enter_context(tc.tile_pool(name="bcp", bufs=2))
        rc_pool = ctx.enter_context(tc.tile_pool(name="rcp", bufs=1))
        ost_pool = ctx.enter_context(tc.tile_pool(name="ost", bufs=1))
        qTcs = {}

        # scratch for PE warm-up + ACT exp-table preload, both issued into
        # the kernel-entry dead time (first DMA can't even issue before
        # ~8us of framework barrier/setup).
        scratch = const_pool.tile([P, SC], F32)
        nc.vector.memset(scratch[:], 0.0)
        dummy_act = const_pool.tile([1, 1], F32)
        nc.scalar.activation(dummy_act[:], ones_sb[0:1, 0:1], Exp, scale=1.0)

        # startup DMAs, ordered so the first QKV unit's data lands first:
        # hsT chunk 0, then the q/k weight columns in 128-col blocks (block
        # b gates only unit b), then the v columns / other constants.
        wqkvT_sb = wq_pool.tile([P, CC, 3 * E], BF16)
        hst0 = hst_pool.tile([P, CC, SC], BF16, tag="hst", name="hst")
        nc.sync.dma_start(
            hst0[:], hsT.ap()[:, 0:SC].rearrange("(co p) t -> p co t", p=P)
        )
        for b8 in range(2 * E // P):
            nc.sync.dma_start(
                wqkvT_sb[:, :, b8 * P:(b8 + 1) * P],
                wqkvT.ap()[:, b8 * P:(b8 + 1) * P].rearrange(
                    "(co p) d -> p co d", p=P
                ),
            )
        nc.sync.dma_start(
            wqkvT_sb[:, :, 2 * E:3 * E],
            wqkvT.ap()[:, 2 * E:3 * E].rearrange("(co p) d -> p co d", p=P),
        )
        # PE warm-up: ~16 matmuls on zero scratch lift the HAM clock gate
        # (4/8 -> 8/8 after ~3.4us of sustained activity) while the real
        # stream is still DMA-gated, so the first QKV unit runs at 2.4GHz.
        wps = ps_all.tile([P, 2, SC], F32, tag="ps2", name="warm")
        for r in range(16):
            nc.tensor.matmul(
                wps[:, r % 2], scratch[:, 0:P].bitcast(F32R),
                scratch[:].bitcast(F32R), start=True, stop=True,
            )
        # non-startup-critical loads, after the phase-1 gating DMAs
        nc.sync.dma_start(vbias_sb[:], vbias.ap())
        nc.sync.dma_start(masks_sb[:], masks.ap())
        woT_sb = wo_pool.tile([P, E // P, C], F32R)
        nc.sync.dma_start(woT_sb[:], woT.ap().rearrange("(ec p) co -> p ec co", p=P))

        def qkv_units(t4):
            st = {}

            def u_load():
                if t4 == 0:
                    st["hst"] = hst0
                else:
                    hst = hst_pool.tile([P, CC, SC], BF16, tag="hst", name="hst")
                    nc.sync.dma_start(
                        hst[:],
                        hsT.ap()[:, t4 * SC:(t4 + 1) * SC].rearrange(
                            "(co p) t -> p co t", p=P
                        ),
                    )
                    st["hst"] = hst
                qTcs[t4] = qt_pool.tile([P, 4, SC], F32R, tag="qTc", name="qTc")

            def u_qk_half(qk, dp, half):
                if "hst" not in st:
                    u_load()
                hst = st["hst"]
                if half == 0:
                    st["ps"] = ps_all.tile([P, 2, SC], F32, tag="ps2", name="psq")
                ps = st["ps"]
                w0 = qk * E + (2 * dp + half) * P
                for cc in range(CC):
                    nc.tensor.matmul(
                        ps[:, half], wqkvT_sb[:, cc, w0:w0 + P], hst[:, cc, :],
                        start=(cc == 0), stop=(cc == CC - 1),
                    )
                if half == 1:
                    dst = (qTcs[t4][:, 2 * dp:2 * dp + 2, :] if qk == 0 else
                           kT[:, 2 * dp:2 * dp + 2, t4 * SC:(t4 + 1) * SC])
                    nc.vector.tensor_add(
                        dst, ps[:],
                        bqkv_sb[:, qk * 4 + 2 * dp:qk * 4 + 2 * dp + 2, None]
                        .to_broadcast((P, 2, SC)),
                    )

            def u_v_half(tp, half):
                hst = st["hst"]
                if half == 0:
                    st["ps"] = ps_all.tile([P, 2, SC], F32, tag="ps2", name="psv")
                ps = st["ps"]
                ts = 2 * tp + half
                for cc in range(CC):
                    nc.tensor.matmul(
                        ps[:, half], hst[:, cc, ts * P:(ts + 1) * P],
                        wqkvT_sb[:, cc, 2 * E:3 * E],
                        start=(cc == 0), stop=(cc == CC - 1),
                    )
                if half == 1:
                    jc = t4 * 4 + 2 * tp
                    nc.vector.tensor_add(
                        v_aug[:, jc:jc + 2, :, 0:HD],
                        ps[:].rearrange("p two (h d) -> p two h d", d=HD),
                        vbias_sb[:, None].rearrange(
                            "p two (h d) -> p two h d", d=HD
                        ).to_broadcast((P, 2, HPC, HD)),
                    )

            units = (
                [lambda qk=qk, dp=dp, h=h: u_qk_half(qk, dp, h)
                 for qk in (0, 1) for dp in (0, 1) for h in (0, 1)]
                + [lambda tp=tp, h=h: u_v_half(tp, h)
                   for tp in (0, 1) for h in (0, 1)]
            )
            return u_load, units

        def emit_qkv(t4):
            u_load, units = qkv_units(t4)
            u_load()
            for u in units:
                u()

        # PV software pipeline carried ACROSS hp and qc boundaries: each
        # pend item owns its (out_ps pair, hp, attnp) so the drain of head-
        # pair n overlaps the score/exp fill of head-pair n+1 and the PE
        # never idles at a boundary (idle gaps also re-throttle the HAM
        # clock gate at 2x cost). The normalize chain for a head-pair is
        # emitted right after its last PV pops.
        apends = []

        def emit_pv(item):
            jc, n0, e, hp, nj, out_ps, attnp = item
            for s in range(2):
                nc.tensor.matmul(
                    out_ps[s][:, n0:SC], v_aug[:, jc, 2 * hp + s, :],
                    e[:, s, n0:SC],
                    start=(jc == 0), stop=(jc == nj - 1),
                )
            if jc == nj - 1:
                emit_norm(hp, out_ps, attnp)

        def emit_norm(hp, out_ps, attnp):
            # normalize by the ones-row sum (reciprocal straight from PSUM,
            # broadcast across partitions via gpsimd) into attnp
            for s in range(2):
                srow = rc_pool.tile([1, SC], F32, tag="srow")
                nc.vector.reciprocal_approx_fast(
                    srow[:], out_ps[s][HD:HD + 1, :]
                )
                bc = bc_pool.tile([64, SC], F32, tag="bc")
                nc.gpsimd.partition_broadcast(bc[:], srow[:])
                nc.vector.tensor_mul(
                    attnp[64 * s:64 * s + 64, hp, :], out_ps[s][0:HD, :], bc[:]
                )

        def emit_attn(qc, hp, attnp, step_hook=None):
            nj = 4 * (qc + 1)
            out_ps = [
                ps_out.tile([HD + 1, SC], F32, tag="outps", name=f"outps{s}")
                for s in range(2)
            ]
            for jc in range(nj):
                di = jc - 4 * qc  # >= 0 on diagonal-straddling chunks
                n0 = P * di if di >= 0 else 0
                j0 = jc * P
                sc_ps = ps_all.tile([P, 2, SC], F32, tag="ps2", name="scps")
                for s in range(2):
                    nc.tensor.matmul(
                        sc_ps[:, s, n0:SC],
                        kT[64 * s:64 * s + 64, hp, j0:j0 + P],
                        qTcs[qc][64 * s:64 * s + 64, hp, n0:SC],
                        start=True, stop=True, tile_position=(64 * s, 0),
                    )
                if len(apends) >= 2:
                    emit_pv(apends.pop(0))
                if step_hook is not None:
                    step_hook()
                e = exp_pool.tile([P, 2, SC], F32R, tag="exp")
                nc.scalar.activation(
                    e[:, :, n0:SC], sc_ps[:, :, n0:SC], Exp, scale=SCALE
                )
                if di >= 0:
                    off = (0, 512, 896, 1152)[di]
                    nc.vector.tensor_mul(
                        e[:, :, n0:SC], e[:, :, n0:SC].bitcast(F32),
                        masks_sb[:, None, off:off + SC - n0]
                        .to_broadcast((P, 2, SC - n0)),
                    )
                apends.append((jc, n0, e, hp, nj, out_ps, attnp))

        def oproj_unit(qc, attnp, t8):
            # both co halves in one 2-bank psum tile -> one copy, one
            # contiguous-row DMA
            trow = qc * SC + t8 * P
            po = ps_all.tile([P, 2, SC], F32, tag="ps2", name="psop")
            for co in range(2):
                for ec in range(E // P):
                    nc.tensor.matmul(
                        po[:, co], attnp[:, ec, t8 * P:(t8 + 1) * P],
                        woT_sb[:, ec, co * SC:(co + 1) * SC],
                        start=(ec == 0), stop=(ec == E // P - 1),
                    )
            st = ost_pool.tile([P, 2, SC], F32, tag="ost")
            nc.vector.tensor_copy(st[:], po[:])
            nc.sync.dma_start(outp.ap()[trow:trow + P, :], st[:])

        def emit_oproj(qc, attnp):
            for t8 in range(SC // P):
                oproj_unit(qc, attnp, t8)

        # software-pipelined emission: QKV for chunk t4+1 and the previous
        # q-chunk's output projection are split into 8-matmul micro-units
        # and dripped into the attention j-loop (the PE is in-order, so
        # filler work must sit between attention steps to absorb the
        # ACT-paced exp lag).
        u_load0, units0 = qkv_units(0)
        u_load0()
        for u in units0[:8]:
            u()  # q and k of chunk 0; the v units ride inside qc0's stream
        carry = units0[8:]
        attnps = {}
        for qc in range(NQC):
            attnps[qc] = attnp_pool.tile(
                [P, E // P, SC], F32R, tag="attnp", name="attnp"
            )
            fillers = list(carry)
            carry = []
            if qc + 1 < NQC:
                u_load, units = qkv_units(qc + 1)
                u_load()  # issue the hsT chunk DMA as early as possible
                fillers.extend(units)
            if qc > 0:
                fillers.extend(
                    (lambda t8=t8: oproj_unit(qc - 1, attnps[qc - 1], t8))
                    for t8 in range(SC // P)
                )
            steps_total = 4 * 4 * (qc + 1)
            nun = len(fillers)
            # start fillers a few steps in: the hoisted hsT DMA needs ~4us
            # (it waits for the previous chunk's last reader) before the
            # first QKV filler's matmuls can run
            state = {"step": 0, "done": 0}

            def step_hook():
                state["step"] += 1
                while (state["done"] < nun
                       and state["step"] * nun >= (state["done"] + 1) * steps_total):
                    fillers[state["done"]]()
                    state["done"] += 1

            for hp in range(4):
                emit_attn(qc, hp, attnps[qc], step_hook)
            for u in fillers[state["done"]:]:
                u()
        while apends:
            emit_pv(apends.pop(0))
        emit_oproj(NQC - 1, attnps[NQC - 1])

    nc.compile()
    return nc


def _prep_inputs(hidden_states, qkv_w, qkv_b, o_w, o_b):
    hidden_states = np.asarray(hidden_states, dtype=np.float32)
    qkv_w = np.asarray(qkv_w, dtype=np.float32)
    qkv_b = np.asarray(qkv_b, dtype=np.float32)
    o_w = np.asarray(o_w, dtype=np.float32)

    import ml_dtypes
    msk = np.zeros((P, 1280), dtype=ml_dtypes.bfloat16)
    j = np.arange(P)[:, None]
    offs = (0, 512, 896, 1152)
    for i in range(4):
        n0 = P * i
        q = np.arange(n0, SC)[None, :]
        msk[:, offs[i]:offs[i] + SC - n0] = ((P * i + j) <= q).astype(ml_dtypes.bfloat16)

    in_maps = []
    for c in range(NCORES):
        b, g = c // 2, c % 2
        hsT = np.ascontiguousarray(hidden_states[b].T.astype(ml_dtypes.bfloat16))
        qsel = qkv_w[E * g:E * g + E]
        ksel = qkv_w[C + E * g:C + E * g + E]
        vsel = qkv_w[2 * C + E * g:2 * C + E * g + E]
        wqkvT = np.ascontiguousarray(
            np.concatenate([qsel, ksel, vsel], 0).T.astype(ml_dtypes.bfloat16)
        )
        woT = np.ascontiguousarray(o_w[:, E * g:E * g + E].T)
        bq = qkv_b[E * g:E * g + E].reshape(4, P).T
        bk = qkv_b[C + E * g:C + E * g + E].reshape(4, P).T
        bv = qkv_b[2 * C + E * g:2 * C + E * g + E]
        bqkv = np.ascontiguousarray(np.concatenate([bq, bk], 1))
        vbias = np.ascontiguousarray(np.tile(bv[None, :], (P, 1)))
        in_maps.append({
            "hsT": hsT, "wqkvT": wqkvT, "woT": woT,
            "bqkv": bqkv, "vbias": vbias, "masks": msk,
        })
    return in_maps


def _get_nc():
    if "nc" not in _cache:
        _cache["nc"] = _build()
    return _cache["nc"]


def _run(in_maps, **kwargs):
    return run_bass_kernel_spmd(
        _get_nc(), in_maps, core_ids=list(range(NCORES)), **kwargs
    )


def kernel(hidden_states, qkv_w, qkv_b, o_w, o_b, **_):
    in_maps = _prep_inputs(hidden_states, qkv_w, qkv_b, o_w, o_b)
    res = _run(in_maps)
    o_b = np.asarray(o_b, dtype=np.float32)
    out = np.empty((B, T, C), dtype=np.float32)
    for b in range(B):
        out[b] = res.results[2 * b]["outp"] + res.results[2 * b + 1]["outp"] + o_b
    return out

